# revision 5
# baseline (speedup 1.0000x reference)
"""CoordinatesToSpikes on 8 TRN2 NeuronCores.

Reference semantics: times = T_EARLY + cv * (T_LATE - T_EARLY);
idx = round(times / DT); spikes = one-hot along a dense time axis of
length 1000 (each (b, c) pair scatters exactly one 1.0, so the scatter
is a pure one-hot materialization: out[b, t, c] = (idx[b, c] == t)).

Strategy (data-parallel over batch, 256 -> 8 x 32):
  - Host computes idx bit-exactly in fp32 (tiny: 64K elements). This
    sidesteps fp32-division semantics on device; values are exact
    integers in [2, 800] representable in fp32.
  - Each core materializes its (32, 1000, 256) f32 shard: idx rows are
    broadcast across partitions with a K=1 PE matmul (ones outer idx),
    then for each 125-row time chunk the DVE computes
    (idx - p) == 125*k via one tensor_scalar, giving the one-hot tile
    [125 time-partitions, 8 batches x 256 channels] which is DMA-stored
    as a single ~1 MB contiguous-per-row transfer.
  - Output is write-only, 32.8 MB per core => memory(store)-roofline.
"""

import numpy as np
from contextlib import ExitStack

import concourse.bass as bass
import concourse.tile as tile
from concourse import bacc, mybir
from concourse.bass_utils import run_bass_kernel_spmd

F32 = mybir.dt.float32

B, C, SEQ = 256, 256, 1000
NCORES = 8
BSH = B // NCORES          # 32 batches per core
GB = 8                     # batches per broadcast group
GROUPS = BSH // GB         # 4
CP = 125                   # time rows (partitions) per chunk
CH = SEQ // CP             # 8 chunks
N = GB * C                 # 2048 free elements per tile

T_EARLY = np.float32(2e-06)
T_LATE_MINUS_EARLY = np.float32(0.0008 - 2e-06)
DT = np.float32(1e-06)

_compiled = None


def _build():
    nc = bacc.Bacc("TRN2", target_bir_lowering=False, debug=False,
                   num_devices=NCORES)
    idx_d = nc.dram_tensor("idx", [1, BSH * C], F32, kind="ExternalInput")
    iota_d = nc.dram_tensor("iota", [CP, 1], F32, kind="ExternalInput")
    out_d = nc.dram_tensor("out", [BSH, SEQ, C], F32, kind="ExternalOutput")

    with ExitStack() as ctx:
        tc = ctx.enter_context(tile.TileContext(nc))
        const = ctx.enter_context(tc.tile_pool(name="const", bufs=1))
        bcpool = ctx.enter_context(tc.tile_pool(name="bc", bufs=2))
        pspool = ctx.enter_context(
            tc.tile_pool(name="ps", bufs=2, space="PSUM"))
        outp = ctx.enter_context(tc.tile_pool(name="outp", bufs=4))

        idx_sb = const.tile([1, BSH * C], F32)
        nc.sync.dma_start(idx_sb[:], idx_d.ap())
        iota_sb = const.tile([CP, 1], F32)
        nc.sync.dma_start(iota_sb[:], iota_d.ap())
        ones_sb = const.tile([1, 128], F32)
        nc.vector.memset(ones_sb[:], 1.0)

        for g in range(GROUPS):
            # Broadcast this group's 2048 idx values to all 128 partitions:
            # ones[1,128].T @ idx[1,2048] -> psum[128,2048], one bank per
            # 512-column matmul.
            ps = pspool.tile([128, N], F32)
            for j in range(N // 512):
                nc.tensor.matmul(
                    ps[:, j * 512:(j + 1) * 512], ones_sb[:],
                    idx_sb[0:1, g * N + j * 512: g * N + (j + 1) * 512],
                    start=True, stop=True)
            bc = bcpool.tile([128, N], F32)
            nc.scalar.copy(bc[:], ps[:])

            for k in range(CH):
                ot = outp.tile([CP, N], F32)
                # one-hot: (idx - p) == 125*k  <=>  idx == 125*k + p
                nc.vector.tensor_scalar(
                    ot[:], bc[0:CP, :], iota_sb[:], float(CP * k),
                    mybir.AluOpType.subtract, mybir.AluOpType.is_equal)
                nc.sync.dma_start(
                    out_d.ap()[g * GB:(g + 1) * GB,
                               k * CP:(k + 1) * CP, :].rearrange(
                                   "b t c -> t b c"),
                    ot[:].rearrange("t (b c) -> t b c", b=GB))
    nc.compile()
    return nc


def _host_idx(coordinate_values: np.ndarray) -> np.ndarray:
    """Bit-exact fp32 mirror of the reference index computation."""
    cv = np.ascontiguousarray(coordinate_values, dtype=np.float32)
    times = T_EARLY + cv * T_LATE_MINUS_EARLY
    return np.rint(times / DT).astype(np.float32)


def kernel(coordinate_values: np.ndarray) -> np.ndarray:
    global _compiled
    if _compiled is None:
        _compiled = _build()
    nc = _compiled

    idxf = _host_idx(coordinate_values)                      # (256, 256)
    iota = np.arange(CP, dtype=np.float32).reshape(CP, 1)
    in_maps = [
        {"idx": np.ascontiguousarray(
            idxf[m * BSH:(m + 1) * BSH].reshape(1, BSH * C)),
         "iota": iota}
        for m in range(NCORES)
    ]
    res = run_bass_kernel_spmd(nc, in_maps, core_ids=list(range(NCORES)))
    return np.concatenate([r["out"] for r in res.results], axis=0)


# revision 7
# speedup vs baseline: 2.7759x; 2.7759x over previous
"""CoordinatesToSpikes on 8 TRN2 NeuronCores.

Reference semantics: times = T_EARLY + cv * (T_LATE - T_EARLY);
idx = round(times / DT); spikes = one-hot along a dense time axis of
length 1000 (each (b, c) pair scatters exactly one 1.0, so the scatter
is a pure one-hot materialization: out[b, t, c] = (idx[b, c] == t)).

Strategy (data-parallel over batch, 256 -> 8 x 32):
  - Host computes idx bit-exactly in fp32 (tiny: 64K elements); values
    are exact integers in [2, 800].
  - On device, SBUF partition p covers batch b = p//4, time-quarter
    tg = p%4 (250 time rows each) so every partition's slice of the
    output is one contiguous 250KB DRAM range -> 10KB DMA descriptors
    (1KB descriptors cap a single HWDGE ring at ~115 GB/s; 10KB ones
    are SDMA-engine-bound at full rate).
  - One K=34 PE matmul builds diff[p, f] = idx[b, f%256] - tg*250
    - f//256 for all partitions (selector rows + folded time base).
    - Then each of 25 chunks (10 time rows) is one DVE compare
    diff == 10*d producing the one-hot tile [128, 2560], DMA-stored as
    a 1.25MB transfer with 10KB contiguous per partition, alternating
    between the two HWDGE rings (sync + scalar engines).
  - Output is write-only, 32.8 MB per core => memory(store)-roofline
    (~92us at 358 GB/s per-core HBM).
"""

import numpy as np
from contextlib import ExitStack

import concourse.bass as bass
import concourse.tile as tile
from concourse import bacc, mybir
from concourse.bass_utils import run_bass_kernel_spmd

F32 = mybir.dt.float32

B, C, SEQ = 256, 256, 1000
NCORES = 8
BSH = B // NCORES          # 32 batches per core
TG = 4                     # time quarters per batch (partition = b*4+tg)
TQ = SEQ // TG             # 250 time rows per quarter
TROWS = 10                 # time rows per chunk
ND = TQ // TROWS           # 25 chunks
FREE = TROWS * C           # 2560 free elements per tile (10KB)
K = BSH + 2                # matmul contraction: 32 selector rows + 2 aux

T_EARLY = np.float32(2e-06)
T_LATE_MINUS_EARLY = np.float32(0.0008 - 2e-06)
DT = np.float32(1e-06)

_compiled = None


def _build():
    nc = bacc.Bacc("TRN2", target_bir_lowering=False, debug=False,
                   num_devices=NCORES)
    idx_d = nc.dram_tensor("idx", [BSH, C], F32, kind="ExternalInput")
    mat_d = nc.dram_tensor("mat", [K, 128], F32, kind="ExternalInput")
    aux_d = nc.dram_tensor("aux", [2, FREE], F32, kind="ExternalInput")
    out_d = nc.dram_tensor("out", [BSH, SEQ, C], F32, kind="ExternalOutput")
    # [128 partitions (b,tg) @ 250KB stride, 25 chunks, 2560 contiguous]
    out_v = out_d.ap().rearrange(
        "b (tg d t) c -> (b tg) d (t c)", tg=TG, d=ND, t=TROWS)

    with ExitStack() as ctx:
        tc = ctx.enter_context(tile.TileContext(nc))
        const = ctx.enter_context(tc.tile_pool(name="const", bufs=1))
        dpool = ctx.enter_context(tc.tile_pool(name="diff", bufs=1))
        pspool = ctx.enter_context(
            tc.tile_pool(name="ps", bufs=1, space="PSUM"))
        outp = ctx.enter_context(tc.tile_pool(name="outp", bufs=4))

        # rhs rows 0..31: idx rows tiled 10x along free; rows 32,33: aux
        # (t_local pattern, ones).
        rhs = const.tile([K, FREE], F32)
        nc.gpsimd.dma_start(
            rhs[0:BSH, :].rearrange("k (r c) -> k r c", r=TROWS),
            idx_d.ap().unsqueeze(1).broadcast_to((BSH, TROWS, C)))
        nc.gpsimd.dma_start(rhs[BSH:K, :], aux_d.ap())
        mat = const.tile([K, 128], F32)
        nc.gpsimd.dma_start(mat[:], mat_d.ap())

        # diff[p, f] = idx[p//4, f%256] - (p%4)*250 - f//256
        ps = pspool.tile([128, FREE], F32)
        for j in range(FREE // 512):
            nc.tensor.matmul(ps[:, j * 512:(j + 1) * 512], mat[:],
                             rhs[:, j * 512:(j + 1) * 512],
                             start=True, stop=True)
        diff = dpool.tile([128, FREE], F32)
        nc.scalar.copy(diff[:], ps[:])

        for d in range(ND):
            ot = outp.tile([128, FREE], F32)
            nc.vector.tensor_scalar(
                ot[:], diff[:], float(TROWS * d), None,
                mybir.AluOpType.is_equal)
            eng = nc.sync if d % 2 == 0 else nc.scalar
            eng.dma_start(out_v[:, d, :], ot[:])
    nc.compile()
    return nc


def _host_idx(coordinate_values: np.ndarray) -> np.ndarray:
    """Bit-exact fp32 mirror of the reference index computation."""
    cv = np.ascontiguousarray(coordinate_values, dtype=np.float32)
    times = T_EARLY + cv * T_LATE_MINUS_EARLY
    return np.rint(times / DT).astype(np.float32)


def _host_consts():
    p = np.arange(128)
    mat = np.zeros((K, 128), np.float32)
    mat[p // TG, p] = 1.0                      # selector rows
    mat[BSH, :] = -1.0                         # coefficient for t_local
    mat[BSH + 1, :] = -(p % TG).astype(np.float32) * TQ  # -tg*250
    aux = np.empty((2, FREE), np.float32)
    aux[0] = np.repeat(np.arange(TROWS, dtype=np.float32), C)  # f//256
    aux[1] = 1.0
    return mat, aux


def _in_maps(coordinate_values: np.ndarray) -> list[dict]:
    idxf = _host_idx(coordinate_values)                      # (256, 256)
    mat, aux = _host_consts()
    return [
        {"idx": np.ascontiguousarray(idxf[m * BSH:(m + 1) * BSH]),
         "mat": mat, "aux": aux}
        for m in range(NCORES)
    ]


def kernel(coordinate_values: np.ndarray) -> np.ndarray:
    global _compiled
    if _compiled is None:
        _compiled = _build()
    res = run_bass_kernel_spmd(
        _compiled, _in_maps(coordinate_values),
        core_ids=list(range(NCORES)))
    return np.concatenate([r["out"] for r in res.results], axis=0)
